# revision 53
# baseline (speedup 1.0000x reference)
"""Trainium2 Bass kernel for nn_ChaosTransformer_22333829939822.

Key mathematical reduction (verified against the reference):
the torch-style ``view(B, H, L, E//H)`` on a [B, L, E] tensor is a raw
row-major reshape, which makes head h attend only within the 256-position
block [h*256, (h+1)*256).  The output ``dec[:, -96:, 0]`` therefore depends
only on the last 256 positions of each batch.  Each core runs one batch's
[256, 256] residual-stream transformer; attention operates on the
[2048, 32] head-view of the 256x256 block.

Sharding: data-parallel over batch B across 4 of the 8 cores (one batch
per core, fully independent, no collectives).

v1 performance rework (from the 276us baseline trace):
- all weights ship in one bf16 blob + one small f32 blob (few large DMAs
  at fabric rate instead of ~100 descriptor-bound transfers).
- decay tiles D[h][32j+d, q] and all rearranged biases precomputed on host
  (the on-device build was 65k 4-byte DMA packets = 45us of dead time).
- LN gain/bias broadcast tiles built by the idle GpSimd engine.
- softmax row sums accumulated as M=32 ones-matmuls (same PE cost as M=1,
  yields the 32-row broadcast for free); 1/RS via reciprocal_approx_fast.
- score matmuls stream N=512 (2-4 score tiles per matmul) into 2-bank PSUM
  regions; ONE exp ACTIVATE per region amortizes ACT's 352-cycle fixed
  overhead (exp is the bottleneck engine: 1 elem/cyc/lane @ 1.2 GHz).
"""

import sys
import numpy as np

sys.path.insert(0, "/opt/trn_rl_repo")

import concourse.bass as bass
import concourse.tile as tile
from concourse import mybir
from concourse.masks import make_identity

F32 = mybir.dt.float32
BF16 = mybir.dt.bfloat16
WDT = BF16
ADD = mybir.AluOpType.add
SUB = mybir.AluOpType.subtract
MULT = mybir.AluOpType.mult
MAX = mybir.AluOpType.max
AF = mybir.ActivationFunctionType

B, L, D, E, DFF, LYR, PRED = 4, 2048, 7, 256, 1024, 2, 96
FACTOR = 5.0
SCALE = 1.0 / float(np.sqrt(FACTOR))
EPS = 1e-5
P0 = L - 256          # 1792: start of the last 256-position block
QLO2 = 208            # layer-2 computes this core's output positions
NPOS = 256
OUTW = NPOS - QLO2    # 48 output positions per core
ROLL = 48             # odd cores get inputs position-rolled by +48, so
                      # their local [QLO2, 256) window is global [160, 208)


# ---------------- blob layouts (host + device share these) ----------------

def _bf16_layout():
    """Column offsets into the [128, C] bf16 weight blob."""
    off = {}
    c = 0
    def put(key, w):
        nonlocal c
        off[key] = c
        c += w
    for r in range(3):
        put(("Prot", r), 128)
    for l in range(LYR):
        for k in range(2):
            put(("Wq", l, k), E)
            put(("Wk", l, k), E)
            put(("Wv", l, k), E)
        for h in range(2):
            put(("Wo", l, h), E)
        for k in range(2):
            put(("W1", l, k), DFF)
        for dk in range(8):
            put(("W2", l, dk), E)
    return off, c


def _f32_layout():
    off = {}
    c = 0
    def put(key, w):
        nonlocal c
        off[key] = c
        c += w
    for h in range(2):
        put(("D", h), NPOS)
    for l in range(LYR):
        put(("bq", l), 2)
        put(("bk", l), 2)
        put(("b1", l), 8)
    put("bemb_pp", 2)
    put("Wp2", 2)
    return off, c


def _rows_layout():
    """f32 rows on partition 0: LN rows + f32 bias rows + bproj."""
    off = {}
    c = 0
    def put(key, w):
        nonlocal c
        off[key] = c
        c += w
    for nm in ("ln1g", "ln1b", "ln2g", "ln2b"):
        for l in range(LYR):
            put((nm, l), E)
    put("bemb_r", E)
    put("bproj", 1)
    return off, c


def _wrows_layout():
    """bf16 bias rows on partition 0 (seed rows for psum bias init)."""
    off = {}
    c = 0
    def put(key, w):
        nonlocal c
        off[key] = c
        c += w
    for nm in ("bv", "bo", "b2"):
        for l in range(LYR):
            put((nm, l), E)
    return off, c


BF_OFF, BF_COLS = _bf16_layout()
F_OFF, F_COLS = _f32_layout()
R_OFF, R_COLS = _rows_layout()
WR_OFF, WR_COLS = _wrows_layout()


def chaos_kernel(tc, outs, ins):
    import contextlib

    nc = tc.nc
    with contextlib.ExitStack() as ctx:
        _chaos_body(tc, nc, ctx, outs, ins)


def _chaos_body(tc, nc, ctx, outs, ins):
    const = ctx.enter_context(tc.tile_pool(name="const", bufs=1))
    work = ctx.enter_context(tc.tile_pool(name="work", bufs=2))
    atp = ctx.enter_context(tc.tile_pool(name="atp", bufs=12))
    psw = ctx.enter_context(tc.tile_pool(name="psw", bufs=2, space="PSUM"))
    scp = ctx.enter_context(tc.tile_pool(name="scp", bufs=2, space="PSUM"))
    psacc = ctx.enter_context(tc.tile_pool(name="psacc", bufs=1, space="PSUM"))

    dma = nc.sync.dma_start

    # ---------------- constant loads (few big DMAs) ----------------
    fblob = const.tile([128, F_COLS], F32, tag="fblob")
    dma(out=fblob[:], in_=ins["fblob"][:])
    rows = const.tile([1, R_COLS], F32, tag="rows")
    dma(out=rows[:], in_=ins["rows"][:])
    wrows = const.tile([1, WR_COLS], BF16, tag="wrows")
    dma(out=wrows[:], in_=ins["wrows"][:])
    xw = const.tile([D, 2 * NPOS], F32, tag="xw")
    dma(out=xw[:], in_=ins["xw"][:])
    blob = const.tile([128, BF_COLS], BF16, tag="blob")
    # split: Prot + layer-0 QKV first so attention can start early
    s1 = BF_OFF[("Wo", 0, 0)]
    s2 = BF_OFF[("Wq", 1, 0)]
    dma(out=blob[:, :s1], in_=ins["blob"][:, :s1])
    dma(out=blob[:, s1:s2], in_=ins["blob"][:, s1:s2])
    dma(out=blob[:, s2:], in_=ins["blob"][:, s2:])
    xT_sb = xw[:, :NPOS]
    Wemb_sb = xw[:, NPOS:]

    def bf(key):
        w = {"Prot": 128, "W1": DFF}.get(key[0], E)
        return blob[:, BF_OFF[key]:BF_OFF[key] + w]

    def fb(key):
        w = {"D": NPOS, "b1": 8}.get(key[0] if isinstance(key, tuple) else key, 2)
        return fblob[:, F_OFF[key]:F_OFF[key] + w]

    def rrow(key):
        w = 1 if key == "bproj" else E
        return rows[0:1, R_OFF[key]:R_OFF[key] + w]

    def wrow(key):
        return wrows[0:1, WR_OFF[key]:WR_OFF[key] + E]

    # LN gain/bias broadcast tiles via GpSimd (engine is otherwise idle)
    ln_w = 4 * LYR * E
    lnall = const.tile([128, ln_w], F32, tag="lnall")
    for o in range(0, ln_w, 2 * E):
        nc.gpsimd.partition_broadcast(lnall[:, o:o + 2 * E],
                                      rows[0:1, o:o + 2 * E])
    ln_b = {}
    for ni, nm in enumerate(("ln1g", "ln1b", "ln2g", "ln2b")):
        for l in range(LYR):
            o = R_OFF[(nm, l)]
            ln_b[(nm, l)] = lnall[:, o:o + E]

    ident = const.tile([128, 128], F32, tag="ident")
    make_identity(nc, ident[:])
    ones_row = const.tile([1, 128], F32, tag="ones_row")
    nc.vector.memset(ones_row[:], 1.0)
    ones_row_w = const.tile([1, 128], WDT, tag="ones_row_w")
    nc.vector.memset(ones_row_w[:], 1.0)
    eps_t = const.tile([128, 1], F32, tag="eps")
    nc.vector.memset(eps_t[:], EPS)

    def seed_bias(ps_ap, brow_ap, m):
        """PSUM <- bias row broadcast over m partitions (K=1 matmul)."""
        ones = ones_row if brow_ap.dtype == F32 else ones_row_w
        nc.tensor.matmul(ps_ap, ones[0:1, :m], brow_ap, start=True, stop=False)

    def layernorm(x_ap, rows_n, g_b, b_b, out_ap):
        st = work.tile([128, 6], F32, tag="bn_st")
        nc.vector.bn_stats(st[:rows_n], x_ap)
        mv = work.tile([128, 2], F32, tag="bn_mv")
        nc.vector.bn_aggr(mv[:rows_n], st[:rows_n])
        sd = work.tile([128, 1], F32, tag="bn_sd")
        nc.scalar.activation(sd[:rows_n], mv[:rows_n, 1:2], AF.Sqrt,
                             bias=eps_t[:rows_n])
        nc.vector.reciprocal(sd[:rows_n], sd[:rows_n])
        if g_b is None:  # final LN: gain/bias folded into W_proj on host
            nc.vector.tensor_scalar(out_ap, x_ap, mv[:rows_n, 0:1],
                                    sd[:rows_n], SUB, MULT)
            return
        t = work.tile([128, NPOS], F32, tag="ln_t")
        nc.vector.tensor_scalar(t[:rows_n], x_ap, mv[:rows_n, 0:1], sd[:rows_n],
                                SUB, MULT)
        nc.vector.tensor_mul(t[:rows_n], t[:rows_n], g_b[:rows_n])
        nc.vector.tensor_add(out_ap, t[:rows_n], b_b[:rows_n])

    # ---------------- embedding ----------------
    X_t, XT_t = {}, {}
    for p in range(2):  # position-major X
        ps = psw.tile([128, 512], F32, tag="qk")
        seed_bias(ps[:, :E], rrow("bemb_r"), 128)
        nc.tensor.matmul(ps[:, :E], xT_sb[:, p * 128:(p + 1) * 128],
                         Wemb_sb[:], start=False, stop=True)
        t = const.tile([128, NPOS], F32, tag=f"X{p}")
        nc.vector.tensor_copy(t[:], ps[:, :E])
        X_t[p] = t
    for k in range(2):  # channel-major XT
        ps = psw.tile([128, 512], F32, tag="qk")
        nc.tensor.matmul(ps[:, :NPOS], Wemb_sb[:, k * 128:(k + 1) * 128],
                         xT_sb[:], start=True, stop=True)
        t = const.tile([128, NPOS], WDT, tag=f"XT{k}")
        nc.vector.tensor_scalar_add(t[:], ps[:, :NPOS],
                                    fb("bemb_pp")[:, k:k + 1])
        XT_t[k] = t

    # ---------------- transformer layers ----------------
    for l in range(LYR):
        qlo, qhi = (0, NPOS) if l == 0 else (QLO2, NPOS)
        qw = qhi - qlo
        pos_chunks = ([(0, 0, 128, 0), (1, 0, 128, 128)] if l == 0
                      else [(2, 0, qw, 0)])
        # (X-tile idx, row offset in tile, nrows, query-col offset)

        # ---- K projection -> KT channel-major bf16 [128, 256] x2
        KT = {}
        for Jt in range(2):
            ps = psw.tile([128, 512], F32, tag="qk")
            for k in range(2):
                nc.tensor.matmul(
                    ps[:, :NPOS],
                    bf(("Wk", l, k))[:, Jt * 128:(Jt + 1) * 128],
                    XT_t[k][:], start=(k == 0), stop=(k == 1))
            t = work.tile([128, NPOS], BF16, tag=f"KT{Jt}")
            nc.vector.tensor_scalar_add(t[:], ps[:, :NPOS],
                                        fb(("bk", l))[:, Jt:Jt + 1])
            KT[Jt] = t

        # ---- V projection -> VO[pc] [128, 8, 64] bf16: per key chunk cp,
        # cols [0:32) = V channels, cols [32:64) = ones.  The combined
        # [V | 1] stationary makes ONE M=64 matmul per exp-region produce
        # both the A@V partial and the softmax row-sum.
        VO = {}
        for pc in range(2):
            ps = psw.tile([128, 512], F32, tag="qk")
            seed_bias(ps[:, :E], wrow(("bv", l)), 128)
            for k in range(2):
                nc.tensor.matmul(
                    ps[:, :E], XT_t[k][:, pc * 128:(pc + 1) * 128],
                    bf(("Wv", l, k))[:], start=False, stop=(k == 1))
            t = work.tile([128, 8, 64], BF16, tag=f"VO{pc}")
            nc.vector.tensor_copy(
                t[:, :, 0:32],
                ps[:, :E].rearrange("p (c d) -> p c d", d=32))
            nc.vector.memset(t[:, :, 32:64], 1.0)
            VO[pc] = t

        # ---- Q projection -> Qs_dbl [128, 2, 2, 4, qw] bf16
        # dims [part, h, dup, r, q]; rotation r written to both dups so a
        # row strip i can read 4 DOUBLED slots starting at (4-i)%4, which
        # enumerates q-chunks c = 4h+0..3 in ascending order.
        Qs_dbl = work.tile([128, 2, 2, 4, qw], BF16, tag=f"qsdbl{l}")
        for h in range(2):
            ps = psw.tile([128, 512], F32, tag="qk")
            for k in range(2):
                nc.tensor.matmul(
                    ps[:, :qw],
                    bf(("Wq", l, k))[:, h * 128:(h + 1) * 128],
                    XT_t[k][:, qlo:qhi], start=(k == 0), stop=(k == 1))
            tf = work.tile([128, NPOS], F32, tag="qtmp")
            nc.vector.tensor_scalar_add(tf[:, :qw], ps[:, :qw],
                                        fb(("bq", l))[:, h:h + 1])
            nc.vector.tensor_mul(Qs_dbl[:, h, 0, 0, :],
                                 tf[:, :qw], fb(("D", h))[:, qlo:qhi])
        for r in range(1, 4):
            for h in range(2):
                ps = psw.tile([128, 512], F32, tag="qk")
                nc.tensor.matmul(ps[:, :qw], bf(("Prot", r - 1))[:],
                                 Qs_dbl[:, h, 0, 0, :],
                                 start=True, stop=True)
                nc.vector.tensor_copy(Qs_dbl[:, h, 0, r, :], ps[:, :qw])
        for h in range(2):  # duplicate the 4 slots (wrap-around reads)
            nc.vector.tensor_copy(Qs_dbl[:, h, 1, :, :], Qs_dbl[:, h, 0, :, :])

        # ---- attention: ST -> exp -> [A@V | rowsum] accumulated in PSUM
        # OTR[64h+d,    c*qw+q] = attention out, q-chunk c=4h+c_local
        # OTR[64h+32+d, c*qw+q] = softmax denominator (identical over d)
        # zeroed by memset; all matmuls accumulate with start=False.
        OTR = psacc.tile([128, 1024], F32, tag="otr")
        nc.vector.memset(OTR[:], 0.0)
        qv = Qs_dbl[:].rearrange("p h u r q -> p (h u r) q")  # [128,16,qw]
        nslot = min(4, 512 // qw)  # c-slots per matmul: 2 for L1, 4 for L2
        ng = 4 // nslot            # score banks per (i,h): 2 for L1, 1 L2
        for J in range(2):          # key c'-quad
            for pc in range(2):     # key position chunk
                # QK: consecutive matmuls walk strips i=2a, 2a+1, ... so
                # the K=32 row-tiles stream concurrently in the PE array.
                AT = {}
                for a in range(2):
                    for h in range(2):
                        for g in range(ng):
                            sc = scp.tile([128, 2, 512], F32, tag="sc")
                            at = atp.tile([128, 2, 512], BF16, tag="at")
                            for b in range(2):
                                i = 2 * a + b
                                s0 = (4 - i) % 4
                                nc.tensor.matmul(
                                    sc[:, b, :nslot * qw],
                                    KT[J][32 * i:32 * (i + 1),
                                          pc * 128:(pc + 1) * 128],
                                    qv[32 * i:32 * (i + 1),
                                       8 * h + s0 + g * nslot:
                                       8 * h + s0 + (g + 1) * nslot, :],
                                    start=True, stop=True,
                                    tile_position=(32 * i, 0))
                            nc.scalar.activation(at[:, :, :nslot * qw],
                                                 sc[:, :, :nslot * qw],
                                                 AF.Exp)
                            AT[(h, g, a)] = at
                # AV + rowsum: one M=64 matmul per psum bank of scores
                for i in range(4):  # key chunk cp = 4J+i
                    cp = 4 * J + i
                    for h in range(2):
                        for o in range(ng):
                            nc.tensor.matmul(
                                OTR[64 * h:64 * h + 64,
                                    o * 512:o * 512 + nslot * qw],
                                VO[pc][:, cp, :],
                                AT[(h, o, i // 2)][:, i % 2, :nslot * qw],
                                start=False, stop=False,
                                skip_group_check=True,
                                tile_position=(0, 64 * h))

        # ---- normalize: OT = OT * (1/RS); redistribute [d,(c,q)] ->
        # [32c+d, q] channel-major via small SBUF->SBUF DMAs.
        otn = work.tile([128, 1024], F32, tag="otn")
        nc.vector.tensor_copy(otn[:, :4 * qw], OTR[:, :4 * qw])
        OT_sb = {}
        for h in range(2):
            ot128 = work.tile([128, NPOS], F32, tag=f"ot128{h}")
            rs128 = work.tile([128, NPOS], F32, tag=f"rs128{h}")
            for c in range(4):
                dma(out=ot128[32 * c:32 * c + 32, :qw],
                    in_=otn[64 * h:64 * h + 32, c * qw:(c + 1) * qw])
                dma(out=rs128[32 * c:32 * c + 32, :qw],
                    in_=otn[64 * h + 32:64 * h + 64, c * qw:(c + 1) * qw])
            rinv = work.tile([128, NPOS], F32, tag=f"rinv{h}")
            nc.vector.reciprocal_approx_fast(rinv[:, :qw], rs128[:, :qw])
            t = work.tile([128, NPOS], WDT, tag=f"OT{h}")
            nc.vector.tensor_tensor(t[:, :qw], ot128[:, :qw],
                                    rinv[:, :qw], MULT)
            OT_sb[h] = t

        # ---- O @ Wo + bo + residual -> LN1 -> xa
        xa = {}
        for ci, (xi, ro, nr, co) in enumerate(pos_chunks):
            ps = psw.tile([128, 512], F32, tag="qk")
            seed_bias(ps[:nr, :E], wrow(("bo", l)), nr)
            for h in range(2):
                nc.tensor.matmul(
                    ps[:nr, :E], OT_sb[h][:, co:co + nr],
                    bf(("Wo", l, h))[:], start=False, stop=(h == 1))
            res = work.tile([128, NPOS], F32, tag=f"res{ci}")
            nc.vector.tensor_add(res[:nr], ps[:nr, :E],
                                 X_t[xi][ro:ro + nr, :])
            t = work.tile([128, NPOS], F32, tag=f"xa{ci}")
            layernorm(res[:nr], nr, ln_b[("ln1g", l)], ln_b[("ln1b", l)],
                      t[:nr])
            xa[ci] = t

        # ---- transpose xa -> xaT channel-major (copies on the idle ACT)
        xaT = {}
        for k in range(2):
            t = work.tile([128, NPOS], WDT, tag=f"xaT{k}")
            for ci, (_, _, nr, co) in enumerate(pos_chunks):
                ps = psw.tile([128, 512], F32, tag="qk")
                nc.tensor.transpose(ps[:, :nr],
                                    xa[ci][:nr, k * 128:(k + 1) * 128],
                                    ident[:nr, :nr])
                nc.vector.tensor_copy(t[:, co:co + nr], ps[:, :nr])
            xaT[k] = t

        # ---- FFN: H1T = relu(W1.T x + b1) channel-major bf16 [128, qw] x8
        H1T = {}
        for dk in range(8):
            ps = psw.tile([128, 512], F32, tag="qk")
            for k in range(2):
                nc.tensor.matmul(
                    ps[:, :qw],
                    bf(("W1", l, k))[:, dk * 128:(dk + 1) * 128],
                    xaT[k][:, :qw], start=(k == 0), stop=(k == 1))
            t = work.tile([128, NPOS], BF16, tag=f"H1T{dk}")
            nc.vector.tensor_scalar(t[:, :qw], ps[:, :qw],
                                    fb(("b1", l))[:, dk:dk + 1], 0.0,
                                    ADD, MAX)
            H1T[dk] = t

        # ---- FF = relu(H1 @ W2 + b2); X_next = LN2(xa + FF)
        newX = {}
        for ci, (_, _, nr, co) in enumerate(pos_chunks):
            ps = psw.tile([128, 512], F32, tag="qk")
            seed_bias(ps[:nr, :E], wrow(("b2", l)), nr)
            for dk in range(8):
                nc.tensor.matmul(
                    ps[:nr, :E], H1T[dk][:, co:co + nr],
                    bf(("W2", l, dk))[:], start=False, stop=(dk == 7))
            t = work.tile([128, NPOS], F32, tag=f"ff{ci}")
            nc.vector.tensor_scalar_max(t[:nr], ps[:nr, :E], 0.0)
            res2 = work.tile([128, NPOS], F32, tag=f"res2{ci}")
            nc.vector.tensor_add(res2[:nr], t[:nr], xa[ci][:nr])
            xn = const.tile([128, NPOS], F32, tag=f"Xn{l}{ci}")
            layernorm(res2[:nr], nr, ln_b[("ln2g", l)], ln_b[("ln2b", l)],
                      xn[:nr])
            newX[ci] = xn

        if l == 0:
            # DVE can't read >32 partitions at a nonzero base: shift the
            # output positions [160,256) to partition base 0 for the L2
            # residual add.
            x2res = const.tile([128, NPOS], F32, tag="x2res")
            dma(out=x2res[0:NPOS - QLO2, :],
                in_=newX[1][QLO2 - 128:128, :])
            X_t = {0: newX[0], 1: newX[1], 2: x2res}
            XT_t = {}
            for k in range(2):
                t = const.tile([128, NPOS], WDT, tag=f"X1T{k}")
                for ci in range(2):
                    ps = psw.tile([128, 512], F32, tag="qk")
                    nc.tensor.transpose(ps[:, :128],
                                        newX[ci][:, k * 128:(k + 1) * 128],
                                        ident[:])
                    nc.vector.tensor_copy(t[:, ci * 128:(ci + 1) * 128],
                                          ps[:, :128])
                XT_t[k] = t
        else:
            X2 = newX[0]  # [PRED, 256]

    # ---------------- final LN + projection ----------------
    xf = work.tile([128, NPOS], F32, tag="xf")
    layernorm(X2[:OUTW], OUTW, None, None, xf[:OUTW])
    xfT = {}
    for k in range(2):
        ps = psw.tile([128, 512], F32, tag="qk")
        nc.tensor.transpose(ps[:, :OUTW], xf[:OUTW, k * 128:(k + 1) * 128],
                            ident[:OUTW, :OUTW])
        t = work.tile([128, 128], F32, tag=f"xfT{k}")
        nc.vector.tensor_copy(t[:, :OUTW], ps[:, :OUTW])
        xfT[k] = t
    ps = psw.tile([128, 512], F32, tag="qk")
    nc.tensor.matmul(ps[:OUTW, 0:1], ones_row[0:1, :OUTW], rrow("bproj"),
                     start=True, stop=False)
    for k in range(2):
        nc.tensor.matmul(ps[:OUTW, 0:1], xfT[k][:, :OUTW],
                         fb("Wp2")[:, k:k + 1],
                         start=False, stop=(k == 1))
    ot = work.tile([128, 1], F32, tag="outsb")
    nc.vector.tensor_copy(ot[:OUTW], ps[:OUTW, 0:1])
    nc.sync.dma_start(out=outs["out"][:], in_=ot[:OUTW, :])


# ======================= host side =======================

def _rot_matrices():
    """P_r[k, m] = 1 iff k = 32*((m//32 + r) % 4) + m % 32, r = 1..3."""
    mats = np.zeros((3, 128, 128), np.float32)
    for r in range(1, 4):
        for m in range(128):
            mats[r - 1, 32 * ((m // 32 + r) % 4) + m % 32, m] = 1.0
    return mats


def _make_in_maps(inputs):
    import ml_dtypes
    f = np.float32
    bh = ml_dtypes.bfloat16
    x_enc = np.asarray(inputs["x_enc"], f)
    td = np.asarray(inputs["time_diffs"], f)

    blob = np.zeros((128, BF_COLS), bh)
    rot = _rot_matrices()
    for r in range(3):
        blob[:, BF_OFF[("Prot", r)]:BF_OFF[("Prot", r)] + 128] = rot[r]
    for l in range(LYR):
        for nm in ("Wq", "Wk", "Wv"):
            w = np.asarray(inputs[nm], f)[l]
            for k in range(2):
                blob[:, BF_OFF[(nm, l, k)]:BF_OFF[(nm, l, k)] + E] = \
                    w[k * 128:(k + 1) * 128, :]
        wo = np.asarray(inputs["Wo"], f)[l]
        for h in range(2):
            blob[:, BF_OFF[("Wo", l, h)]:BF_OFF[("Wo", l, h)] + E] = \
                wo[h * 128:(h + 1) * 128, :]
        w1 = np.asarray(inputs["W1"], f)[l]
        for k in range(2):
            blob[:, BF_OFF[("W1", l, k)]:BF_OFF[("W1", l, k)] + DFF] = \
                w1[k * 128:(k + 1) * 128, :]
        w2 = np.asarray(inputs["W2"], f)[l]
        for dk in range(8):
            blob[:, BF_OFF[("W2", l, dk)]:BF_OFF[("W2", l, dk)] + E] = \
                w2[dk * 128:(dk + 1) * 128, :]

    fblob_base = np.zeros((128, F_COLS), f)
    for l in range(LYR):
        for nm, w in (("bq", 2), ("bk", 2), ("b1", 8)):
            arr = np.asarray(inputs[nm], f)[l].reshape(w, 128).T
            fblob_base[:, F_OFF[(nm, l)]:F_OFF[(nm, l)] + w] = arr
    fblob_base[:, F_OFF["bemb_pp"]:F_OFF["bemb_pp"] + 2] = \
        np.asarray(inputs["b_emb"], f).reshape(2, 128).T
    fblob_base[:, F_OFF["Wp2"]:F_OFF["Wp2"] + 2] = \
        (np.asarray(inputs["lnf_g"], f)
         * np.asarray(inputs["W_proj"], f)[:, 0]).reshape(2, 128).T

    rows = np.zeros((1, R_COLS), f)
    for nm, src in (("ln1g", "ln1_g"), ("ln1b", "ln1_b"),
                    ("ln2g", "ln2_g"), ("ln2b", "ln2_b")):
        for l in range(LYR):
            rows[0, R_OFF[(nm, l)]:R_OFF[(nm, l)] + E] = \
                np.asarray(inputs[src], f)[l]
    rows[0, R_OFF["bemb_r"]:R_OFF["bemb_r"] + E] = \
        np.asarray(inputs["b_emb"], f)
    # final LN gain/bias folded into the projection column:
    # (z*g + b) @ Wp0 + bp0 = z @ (g*Wp0) + (b@Wp0 + bp0)
    wp0 = np.asarray(inputs["W_proj"], f)[:, 0]
    lnfg = np.asarray(inputs["lnf_g"], f)
    lnfb = np.asarray(inputs["lnf_b"], f)
    rows[0, R_OFF["bproj"]] = (np.asarray(inputs["b_proj"], f)[0]
                               + float(lnfb @ wp0))

    wrows = np.zeros((1, WR_COLS), bh)
    for nm, src in (("bv", "bv"), ("bo", "bo"), ("b2", "b2")):
        for l in range(LYR):
            wrows[0, WR_OFF[(nm, l)]:WR_OFF[(nm, l)] + E] = \
                np.asarray(inputs[src], f)[l]

    # 8 cores: 2 per batch.  Core 2b handles output positions [208, 256);
    # core 2b+1 gets inputs position-rolled by +ROLL so its local window
    # [208, 256) is global [160, 208).  Layer-1 is position-wise or
    # key-order-invariant, so the roll only permutes it.
    maps = []
    for b in range(B):
        dec0 = SCALE * np.exp(-td[b].reshape(NPOS, 8) / FACTOR)  # [q, chunk]
        x0 = x_enc[b, P0:P0 + NPOS, :]                           # [q, D]
        for roll in (0, ROLL):
            fblob = fblob_base.copy()
            dec = np.roll(dec0, roll, axis=0)
            for h in range(2):
                tile_ = np.repeat(dec[:, 4 * h:4 * h + 4].T, 32, axis=0)
                fblob[:, F_OFF[("D", h)]:F_OFF[("D", h)] + NPOS] = tile_
            xwa = np.zeros((D, 2 * NPOS), f)
            xwa[:, :NPOS] = np.roll(x0, roll, axis=0).T
            xwa[:, NPOS:] = np.asarray(inputs["W_emb"], f)
            maps.append({
                "blob": blob, "fblob": np.ascontiguousarray(fblob),
                "rows": rows, "wrows": wrows,
                "xw": np.ascontiguousarray(xwa),
            })
    return maps


def _run(in_maps, check_with_sim=False, check_with_hw=True, **kw):
    from concourse.bass_test_utils import run_kernel

    n = len(in_maps)
    out_like = {"out": np.zeros((OUTW, 1), np.float32)}
    res = run_kernel(
        lambda tc, outs, ins: chaos_kernel(tc, outs, ins),
        None,
        in_maps if n > 1 else in_maps[0],
        output_like=[out_like] * n if n > 1 else out_like,
        bass_type=tile.TileContext,
        num_cores=n,
        check_with_sim=check_with_sim,
        check_with_hw=check_with_hw,
        trace_sim=False,
        **kw,
    )
    return res


def kernel(**inputs):
    in_maps = _make_in_maps(inputs)
    res = _run(in_maps)
    # core 2b+1 produced global [160, 208), core 2b produced [208, 256)
    out = np.stack([
        np.concatenate([
            list(res.results[2 * b + 1].values())[0].reshape(OUTW),
            list(res.results[2 * b].values())[0].reshape(OUTW),
        ]) for b in range(B)])
    return out.astype(np.float32)


# revision 54
# speedup vs baseline: 1.3276x; 1.3276x over previous
"""Trainium2 Bass kernel for nn_ChaosTransformer_22333829939822.

Key mathematical reduction (verified against the reference):
the torch-style ``view(B, H, L, E//H)`` on a [B, L, E] tensor is a raw
row-major reshape, which makes head h attend only within the 256-position
block [h*256, (h+1)*256).  The output ``dec[:, -96:, 0]`` therefore depends
only on the last 256 positions of each batch.  Each core runs one batch's
[256, 256] residual-stream transformer; attention operates on the
[2048, 32] head-view of the 256x256 block.

Sharding: data-parallel over batch B across 4 of the 8 cores (one batch
per core, fully independent, no collectives).

v1 performance rework (from the 276us baseline trace):
- all weights ship in one bf16 blob + one small f32 blob (few large DMAs
  at fabric rate instead of ~100 descriptor-bound transfers).
- decay tiles D[h][32j+d, q] and all rearranged biases precomputed on host
  (the on-device build was 65k 4-byte DMA packets = 45us of dead time).
- LN gain/bias broadcast tiles built by the idle GpSimd engine.
- softmax row sums accumulated as M=32 ones-matmuls (same PE cost as M=1,
  yields the 32-row broadcast for free); 1/RS via reciprocal_approx_fast.
- score matmuls stream N=512 (2-4 score tiles per matmul) into 2-bank PSUM
  regions; ONE exp ACTIVATE per region amortizes ACT's 352-cycle fixed
  overhead (exp is the bottleneck engine: 1 elem/cyc/lane @ 1.2 GHz).
"""

import sys
import numpy as np

sys.path.insert(0, "/opt/trn_rl_repo")

import concourse.bass as bass
import concourse.tile as tile
from concourse import mybir
from concourse.masks import make_identity

F32 = mybir.dt.float32
BF16 = mybir.dt.bfloat16
WDT = BF16
ADD = mybir.AluOpType.add
SUB = mybir.AluOpType.subtract
MULT = mybir.AluOpType.mult
MAX = mybir.AluOpType.max
AF = mybir.ActivationFunctionType

B, L, D, E, DFF, LYR, PRED = 4, 2048, 7, 256, 1024, 2, 96
FACTOR = 5.0
SCALE = 1.0 / float(np.sqrt(FACTOR))
EPS = 1e-5
P0 = L - 256          # 1792: start of the last 256-position block
QLO2 = 208            # layer-2 computes this core's output positions
NPOS = 256
OUTW = NPOS - QLO2    # 48 output positions per core
ROLL = 48             # odd cores get inputs position-rolled by +48, so
                      # their local [QLO2, 256) window is global [160, 208)


# ---------------- blob layouts (host + device share these) ----------------

def _bf16_layout():
    """Column offsets into the [128, C] bf16 weight blob."""
    off = {}
    c = 0
    def put(key, w):
        nonlocal c
        off[key] = c
        c += w
    for r in range(3):
        put(("Prot", r), 128)
    for l in range(LYR):
        for k in range(2):
            put(("Wq", l, k), E)
            put(("Wk", l, k), E)
            put(("Wv", l, k), E)
        for h in range(2):
            put(("Wo", l, h), E)
        for k in range(2):
            put(("W1", l, k), DFF)
        for dk in range(8):
            put(("W2", l, dk), E)
    return off, c


def _f32_layout():
    off = {}
    c = 0
    def put(key, w):
        nonlocal c
        off[key] = c
        c += w
    for h in range(2):
        put(("D", h), NPOS)
    for l in range(LYR):
        put(("bq", l), 2)
        put(("bk", l), 2)
        put(("b1", l), 8)
    put("bemb_pp", 2)
    put("Wp2", 2)
    return off, c


def _rows_layout():
    """f32 rows on partition 0: LN rows + f32 bias rows + bproj."""
    off = {}
    c = 0
    def put(key, w):
        nonlocal c
        off[key] = c
        c += w
    for nm in ("ln1g", "ln1b", "ln2g", "ln2b"):
        for l in range(LYR):
            put((nm, l), E)
    put("bemb_r", E)
    put("bproj", 1)
    return off, c


def _wrows_layout():
    """bf16 bias rows on partition 0 (seed rows for psum bias init)."""
    off = {}
    c = 0
    def put(key, w):
        nonlocal c
        off[key] = c
        c += w
    for nm in ("bv", "bo", "b2"):
        for l in range(LYR):
            put((nm, l), E)
    return off, c


BF_OFF, BF_COLS = _bf16_layout()
F_OFF, F_COLS = _f32_layout()
R_OFF, R_COLS = _rows_layout()
WR_OFF, WR_COLS = _wrows_layout()


def chaos_kernel(tc, outs, ins):
    import contextlib

    nc = tc.nc
    with contextlib.ExitStack() as ctx:
        _chaos_body(tc, nc, ctx, outs, ins)


def _chaos_body(tc, nc, ctx, outs, ins):
    const = ctx.enter_context(tc.tile_pool(name="const", bufs=1))
    work = ctx.enter_context(tc.tile_pool(name="work", bufs=2))
    atp = ctx.enter_context(tc.tile_pool(name="atp", bufs=12))
    psw = ctx.enter_context(tc.tile_pool(name="psw", bufs=2, space="PSUM"))
    scp = ctx.enter_context(tc.tile_pool(name="scp", bufs=2, space="PSUM"))
    psacc = ctx.enter_context(tc.tile_pool(name="psacc", bufs=1, space="PSUM"))

    dma = nc.sync.dma_start

    # ---------------- constant loads (few big DMAs) ----------------
    fblob = const.tile([128, F_COLS], F32, tag="fblob")
    dma(out=fblob[:], in_=ins["fblob"][:])
    rows = const.tile([1, R_COLS], F32, tag="rows")
    dma(out=rows[:], in_=ins["rows"][:])
    wrows = const.tile([1, WR_COLS], BF16, tag="wrows")
    dma(out=wrows[:], in_=ins["wrows"][:])
    xw = const.tile([D, 2 * NPOS], F32, tag="xw")
    dma(out=xw[:], in_=ins["xw"][:])
    blob = const.tile([128, BF_COLS], BF16, tag="blob")
    # split: Prot + layer-0 QKV first so attention can start early
    s1 = BF_OFF[("Wo", 0, 0)]
    s2 = BF_OFF[("Wq", 1, 0)]
    dma(out=blob[:, :s1], in_=ins["blob"][:, :s1])
    dma(out=blob[:, s1:s2], in_=ins["blob"][:, s1:s2])
    dma(out=blob[:, s2:], in_=ins["blob"][:, s2:])
    xT_sb = xw[:, :NPOS]
    Wemb_sb = xw[:, NPOS:]

    def bf(key):
        w = {"Prot": 128, "W1": DFF}.get(key[0], E)
        return blob[:, BF_OFF[key]:BF_OFF[key] + w]

    def fb(key):
        w = {"D": NPOS, "b1": 8}.get(key[0] if isinstance(key, tuple) else key, 2)
        return fblob[:, F_OFF[key]:F_OFF[key] + w]

    def rrow(key):
        w = 1 if key == "bproj" else E
        return rows[0:1, R_OFF[key]:R_OFF[key] + w]

    def wrow(key):
        return wrows[0:1, WR_OFF[key]:WR_OFF[key] + E]

    # LN gain/bias broadcast tiles via GpSimd (engine is otherwise idle)
    ln_w = 4 * LYR * E
    lnall = const.tile([128, ln_w], F32, tag="lnall")
    for o in range(0, ln_w, 2 * E):
        nc.gpsimd.partition_broadcast(lnall[:, o:o + 2 * E],
                                      rows[0:1, o:o + 2 * E])
    ln_b = {}
    for ni, nm in enumerate(("ln1g", "ln1b", "ln2g", "ln2b")):
        for l in range(LYR):
            o = R_OFF[(nm, l)]
            ln_b[(nm, l)] = lnall[:, o:o + E]

    ident = const.tile([128, 128], F32, tag="ident")
    make_identity(nc, ident[:])
    ones_row = const.tile([1, 128], F32, tag="ones_row")
    nc.vector.memset(ones_row[:], 1.0)
    ones_row_w = const.tile([1, 128], WDT, tag="ones_row_w")
    nc.vector.memset(ones_row_w[:], 1.0)
    eps_t = const.tile([128, 1], F32, tag="eps")
    nc.vector.memset(eps_t[:], EPS)

    def seed_bias(ps_ap, brow_ap, m):
        """PSUM <- bias row broadcast over m partitions (K=1 matmul)."""
        ones = ones_row if brow_ap.dtype == F32 else ones_row_w
        nc.tensor.matmul(ps_ap, ones[0:1, :m], brow_ap, start=True, stop=False)

    def layernorm(x_ap, rows_n, g_b, b_b, out_ap):
        st = work.tile([128, 6], F32, tag="bn_st")
        nc.vector.bn_stats(st[:rows_n], x_ap)
        mv = work.tile([128, 2], F32, tag="bn_mv")
        nc.vector.bn_aggr(mv[:rows_n], st[:rows_n])
        sd = work.tile([128, 1], F32, tag="bn_sd")
        nc.scalar.activation(sd[:rows_n], mv[:rows_n, 1:2], AF.Sqrt,
                             bias=eps_t[:rows_n])
        nc.vector.reciprocal(sd[:rows_n], sd[:rows_n])
        if g_b is None:  # final LN: gain/bias folded into W_proj on host
            nc.vector.tensor_scalar(out_ap, x_ap, mv[:rows_n, 0:1],
                                    sd[:rows_n], SUB, MULT)
            return
        t = work.tile([128, NPOS], F32, tag="ln_t")
        nc.vector.tensor_scalar(t[:rows_n], x_ap, mv[:rows_n, 0:1], sd[:rows_n],
                                SUB, MULT)
        nc.vector.tensor_mul(t[:rows_n], t[:rows_n], g_b[:rows_n])
        nc.vector.tensor_add(out_ap, t[:rows_n], b_b[:rows_n])

    # ---------------- embedding ----------------
    X_t, XT_t = {}, {}
    for p in range(2):  # position-major X
        ps = psw.tile([128, 512], F32, tag="qk")
        seed_bias(ps[:, :E], rrow("bemb_r"), 128)
        nc.tensor.matmul(ps[:, :E], xT_sb[:, p * 128:(p + 1) * 128],
                         Wemb_sb[:], start=False, stop=True)
        t = const.tile([128, NPOS], F32, tag=f"X{p}")
        nc.vector.tensor_copy(t[:], ps[:, :E])
        X_t[p] = t
    for k in range(2):  # channel-major XT
        ps = psw.tile([128, 512], F32, tag="qk")
        nc.tensor.matmul(ps[:, :NPOS], Wemb_sb[:, k * 128:(k + 1) * 128],
                         xT_sb[:], start=True, stop=True)
        t = const.tile([128, NPOS], WDT, tag=f"XT{k}")
        nc.vector.tensor_scalar_add(t[:], ps[:, :NPOS],
                                    fb("bemb_pp")[:, k:k + 1])
        XT_t[k] = t

    # ---------------- transformer layers ----------------
    for l in range(LYR):
        qlo, qhi = (0, NPOS) if l == 0 else (QLO2, NPOS)
        qw = qhi - qlo
        pos_chunks = ([(0, 0, 128, 0), (1, 0, 128, 128)] if l == 0
                      else [(2, 0, qw, 0)])
        # (X-tile idx, row offset in tile, nrows, query-col offset)

        # ---- K projection -> KT channel-major bf16 [128, 256] x2
        KT = {}
        for Jt in range(2):
            ps = psw.tile([128, 512], F32, tag="qk")
            for k in range(2):
                nc.tensor.matmul(
                    ps[:, :NPOS],
                    bf(("Wk", l, k))[:, Jt * 128:(Jt + 1) * 128],
                    XT_t[k][:], start=(k == 0), stop=(k == 1))
            t = work.tile([128, NPOS], BF16, tag=f"KT{Jt}")
            nc.vector.tensor_scalar_add(t[:], ps[:, :NPOS],
                                        fb(("bk", l))[:, Jt:Jt + 1])
            KT[Jt] = t

        # ---- V projection -> VO[pc] [128, 8, 64] bf16: per key chunk cp,
        # cols [0:32) = V channels, cols [32:64) = ones.  The combined
        # [V | 1] stationary makes ONE M=64 matmul per exp-region produce
        # both the A@V partial and the softmax row-sum.
        VO = {}
        for pc in range(2):
            ps = psw.tile([128, 512], F32, tag="qk")
            seed_bias(ps[:, :E], wrow(("bv", l)), 128)
            for k in range(2):
                nc.tensor.matmul(
                    ps[:, :E], XT_t[k][:, pc * 128:(pc + 1) * 128],
                    bf(("Wv", l, k))[:], start=False, stop=(k == 1))
            t = work.tile([128, 8, 64], BF16, tag=f"VO{pc}")
            nc.vector.tensor_copy(
                t[:, :, 0:32],
                ps[:, :E].rearrange("p (c d) -> p c d", d=32))
            nc.vector.memset(t[:, :, 32:64], 1.0)
            VO[pc] = t

        # ---- Q projection -> Qs_dbl [128, 2, 2, 4, qw] bf16
        # dims [part, h, dup, r, q]; rotation r written to both dups so a
        # row strip i can read 4 DOUBLED slots starting at (4-i)%4, which
        # enumerates q-chunks c = 4h+0..3 in ascending order.
        Qs_dbl = work.tile([128, 2, 2, 4, qw], BF16, tag=f"qsdbl{l}")
        for h in range(2):
            ps = psw.tile([128, 512], F32, tag="qk")
            for k in range(2):
                nc.tensor.matmul(
                    ps[:, :qw],
                    bf(("Wq", l, k))[:, h * 128:(h + 1) * 128],
                    XT_t[k][:, qlo:qhi], start=(k == 0), stop=(k == 1))
            tf = work.tile([128, NPOS], F32, tag="qtmp")
            nc.vector.tensor_scalar_add(tf[:, :qw], ps[:, :qw],
                                        fb(("bq", l))[:, h:h + 1])
            nc.vector.tensor_mul(Qs_dbl[:, h, 0, 0, :],
                                 tf[:, :qw], fb(("D", h))[:, qlo:qhi])
        for r in range(1, 4):
            for h in range(2):
                ps = psw.tile([128, 512], F32, tag="qk")
                nc.tensor.matmul(ps[:, :qw], bf(("Prot", r - 1))[:],
                                 Qs_dbl[:, h, 0, 0, :],
                                 start=True, stop=True)
                nc.vector.tensor_copy(Qs_dbl[:, h, 0, r, :], ps[:, :qw])
        for h in range(2):  # duplicate the 4 slots (wrap-around reads)
            nc.vector.tensor_copy(Qs_dbl[:, h, 1, :, :], Qs_dbl[:, h, 0, :, :])

        # ---- attention: ST -> exp -> [A@V | rowsum] accumulated in PSUM
        # OTR[64h+d,    c*qw+q] = attention out, q-chunk c=4h+c_local
        # OTR[64h+32+d, c*qw+q] = softmax denominator (identical over d)
        # zeroed by memset; all matmuls accumulate with start=False.
        OTR = psacc.tile([128, 1024], F32, tag="otr")
        nc.vector.memset(OTR[:], 0.0)
        qv = Qs_dbl[:].rearrange("p h u r q -> p (h u r) q")  # [128,16,qw]
        nslot = min(4, 512 // qw)  # c-slots per matmul: 2 for L1, 4 for L2
        ng = 4 // nslot            # score banks per (i,h): 2 for L1, 1 L2
        for J in range(2):          # key c'-quad
            for pc in range(2):     # key position chunk
                # QK: consecutive matmuls walk strips i=2a, 2a+1, ... so
                # the K=32 row-tiles stream concurrently in the PE array.
                AT = {}
                for a in range(2):
                    for h in range(2):
                        for g in range(ng):
                            sc = scp.tile([128, 2, 512], F32, tag="sc")
                            at = atp.tile([128, 2, 512], BF16, tag="at")
                            for b in range(2):
                                i = 2 * a + b
                                s0 = (4 - i) % 4
                                nc.tensor.matmul(
                                    sc[:, b, :nslot * qw],
                                    KT[J][32 * i:32 * (i + 1),
                                          pc * 128:(pc + 1) * 128],
                                    qv[32 * i:32 * (i + 1),
                                       8 * h + s0 + g * nslot:
                                       8 * h + s0 + (g + 1) * nslot, :],
                                    start=True, stop=True,
                                    tile_position=(32 * i, 0))
                            nc.scalar.activation(at[:, :, :nslot * qw],
                                                 sc[:, :, :nslot * qw],
                                                 AF.Exp)
                            AT[(h, g, a)] = at
                # AV + rowsum: one M=64 matmul per psum bank of scores
                for i in range(4):  # key chunk cp = 4J+i
                    cp = 4 * J + i
                    for h in range(2):
                        for o in range(ng):
                            nc.tensor.matmul(
                                OTR[64 * h:64 * h + 64,
                                    o * 512:o * 512 + nslot * qw],
                                VO[pc][:, cp, :],
                                AT[(h, o, i // 2)][:, i % 2, :nslot * qw],
                                start=False, stop=False,
                                skip_group_check=True,
                                tile_position=(0, 64 * h))

        # ---- normalize: OT = OT * (1/RS); redistribute [d,(c,q)] ->
        # [32c+d, q] channel-major with identity matmuls (row strip 64h ->
        # col strip 32c), accumulating onto a memset PSUM region.
        otn = work.tile([128, 1024], F32, tag="otn")
        nc.vector.tensor_copy(otn[:, :4 * qw], OTR[:, :4 * qw])
        OT_sb = {}
        for h in range(2):
            rd = scp.tile([128, 2, 512], F32, tag="sc")
            nc.vector.memset(rd[:, :, :qw], 0.0)
            for c in range(4):
                for u in range(2):  # u=0: OT rows, u=1: RS rows
                    nc.tensor.matmul(
                        rd[32 * c:32 * c + 32, u, :qw],
                        ident[64 * h + 32 * u:64 * h + 32 * u + 32,
                              64 * h + 32 * u:64 * h + 32 * u + 32],
                        otn[64 * h + 32 * u:64 * h + 32 * u + 32,
                            c * qw:(c + 1) * qw],
                        start=False, stop=False, skip_group_check=True,
                        tile_position=(64 * h + 32 * u, 32 * c))
            ot128 = work.tile([128, NPOS], F32, tag=f"ot128{h}")
            rs128 = work.tile([128, NPOS], F32, tag=f"rs128{h}")
            nc.vector.tensor_copy(ot128[:, :qw], rd[:, 0, :qw])
            nc.vector.tensor_copy(rs128[:, :qw], rd[:, 1, :qw])
            rinv = work.tile([128, NPOS], F32, tag=f"rinv{h}")
            nc.vector.reciprocal_approx_fast(rinv[:, :qw], rs128[:, :qw])
            t = work.tile([128, NPOS], WDT, tag=f"OT{h}")
            nc.vector.tensor_tensor(t[:, :qw], ot128[:, :qw],
                                    rinv[:, :qw], MULT)
            OT_sb[h] = t

        # ---- O @ Wo + bo + residual -> LN1 -> xa
        xa = {}
        for ci, (xi, ro, nr, co) in enumerate(pos_chunks):
            ps = psw.tile([128, 512], F32, tag="qk")
            seed_bias(ps[:nr, :E], wrow(("bo", l)), nr)
            for h in range(2):
                nc.tensor.matmul(
                    ps[:nr, :E], OT_sb[h][:, co:co + nr],
                    bf(("Wo", l, h))[:], start=False, stop=(h == 1))
            res = work.tile([128, NPOS], F32, tag=f"res{ci}")
            nc.vector.tensor_add(res[:nr], ps[:nr, :E],
                                 X_t[xi][ro:ro + nr, :])
            t = work.tile([128, NPOS], F32, tag=f"xa{ci}")
            layernorm(res[:nr], nr, ln_b[("ln1g", l)], ln_b[("ln1b", l)],
                      t[:nr])
            xa[ci] = t

        # ---- transpose xa -> xaT channel-major (copies on the idle ACT)
        xaT = {}
        for k in range(2):
            t = work.tile([128, NPOS], WDT, tag=f"xaT{k}")
            for ci, (_, _, nr, co) in enumerate(pos_chunks):
                ps = psw.tile([128, 512], F32, tag="qk")
                nc.tensor.transpose(ps[:, :nr],
                                    xa[ci][:nr, k * 128:(k + 1) * 128],
                                    ident[:nr, :nr])
                nc.vector.tensor_copy(t[:, co:co + nr], ps[:, :nr])
            xaT[k] = t

        # ---- FFN: H1T = relu(W1.T x + b1) channel-major bf16 [128, qw] x8
        H1T = {}
        for dk in range(8):
            ps = psw.tile([128, 512], F32, tag="qk")
            for k in range(2):
                nc.tensor.matmul(
                    ps[:, :qw],
                    bf(("W1", l, k))[:, dk * 128:(dk + 1) * 128],
                    xaT[k][:, :qw], start=(k == 0), stop=(k == 1))
            t = work.tile([128, NPOS], BF16, tag=f"H1T{dk}")
            nc.vector.tensor_scalar(t[:, :qw], ps[:, :qw],
                                    fb(("b1", l))[:, dk:dk + 1], 0.0,
                                    ADD, MAX)
            H1T[dk] = t

        # ---- FF = relu(H1 @ W2 + b2); X_next = LN2(xa + FF)
        newX = {}
        for ci, (_, _, nr, co) in enumerate(pos_chunks):
            ps = psw.tile([128, 512], F32, tag="qk")
            seed_bias(ps[:nr, :E], wrow(("b2", l)), nr)
            for dk in range(8):
                nc.tensor.matmul(
                    ps[:nr, :E], H1T[dk][:, co:co + nr],
                    bf(("W2", l, dk))[:], start=False, stop=(dk == 7))
            t = work.tile([128, NPOS], F32, tag=f"ff{ci}")
            nc.vector.tensor_scalar_max(t[:nr], ps[:nr, :E], 0.0)
            res2 = work.tile([128, NPOS], F32, tag=f"res2{ci}")
            nc.vector.tensor_add(res2[:nr], t[:nr], xa[ci][:nr])
            xn = const.tile([128, NPOS], F32, tag=f"Xn{l}{ci}")
            layernorm(res2[:nr], nr, ln_b[("ln2g", l)], ln_b[("ln2b", l)],
                      xn[:nr])
            newX[ci] = xn

        if l == 0:
            # DVE can't read >32 partitions at a nonzero base: shift the
            # output positions [160,256) to partition base 0 for the L2
            # residual add.
            x2res = const.tile([128, NPOS], F32, tag="x2res")
            dma(out=x2res[0:NPOS - QLO2, :],
                in_=newX[1][QLO2 - 128:128, :])
            X_t = {0: newX[0], 1: newX[1], 2: x2res}
            XT_t = {}
            for k in range(2):
                t = const.tile([128, NPOS], WDT, tag=f"X1T{k}")
                for ci in range(2):
                    ps = psw.tile([128, 512], F32, tag="qk")
                    nc.tensor.transpose(ps[:, :128],
                                        newX[ci][:, k * 128:(k + 1) * 128],
                                        ident[:])
                    nc.vector.tensor_copy(t[:, ci * 128:(ci + 1) * 128],
                                          ps[:, :128])
                XT_t[k] = t
        else:
            X2 = newX[0]  # [PRED, 256]

    # ---------------- final LN + projection ----------------
    xf = work.tile([128, NPOS], F32, tag="xf")
    layernorm(X2[:OUTW], OUTW, None, None, xf[:OUTW])
    xfT = {}
    for k in range(2):
        ps = psw.tile([128, 512], F32, tag="qk")
        nc.tensor.transpose(ps[:, :OUTW], xf[:OUTW, k * 128:(k + 1) * 128],
                            ident[:OUTW, :OUTW])
        t = work.tile([128, 128], F32, tag=f"xfT{k}")
        nc.vector.tensor_copy(t[:, :OUTW], ps[:, :OUTW])
        xfT[k] = t
    ps = psw.tile([128, 512], F32, tag="qk")
    nc.tensor.matmul(ps[:OUTW, 0:1], ones_row[0:1, :OUTW], rrow("bproj"),
                     start=True, stop=False)
    for k in range(2):
        nc.tensor.matmul(ps[:OUTW, 0:1], xfT[k][:, :OUTW],
                         fb("Wp2")[:, k:k + 1],
                         start=False, stop=(k == 1))
    ot = work.tile([128, 1], F32, tag="outsb")
    nc.vector.tensor_copy(ot[:OUTW], ps[:OUTW, 0:1])
    nc.sync.dma_start(out=outs["out"][:], in_=ot[:OUTW, :])


# ======================= host side =======================

def _rot_matrices():
    """P_r[k, m] = 1 iff k = 32*((m//32 + r) % 4) + m % 32, r = 1..3."""
    mats = np.zeros((3, 128, 128), np.float32)
    for r in range(1, 4):
        for m in range(128):
            mats[r - 1, 32 * ((m // 32 + r) % 4) + m % 32, m] = 1.0
    return mats


def _make_in_maps(inputs):
    import ml_dtypes
    f = np.float32
    bh = ml_dtypes.bfloat16
    x_enc = np.asarray(inputs["x_enc"], f)
    td = np.asarray(inputs["time_diffs"], f)

    blob = np.zeros((128, BF_COLS), bh)
    rot = _rot_matrices()
    for r in range(3):
        blob[:, BF_OFF[("Prot", r)]:BF_OFF[("Prot", r)] + 128] = rot[r]
    for l in range(LYR):
        for nm in ("Wq", "Wk", "Wv"):
            w = np.asarray(inputs[nm], f)[l]
            for k in range(2):
                blob[:, BF_OFF[(nm, l, k)]:BF_OFF[(nm, l, k)] + E] = \
                    w[k * 128:(k + 1) * 128, :]
        wo = np.asarray(inputs["Wo"], f)[l]
        for h in range(2):
            blob[:, BF_OFF[("Wo", l, h)]:BF_OFF[("Wo", l, h)] + E] = \
                wo[h * 128:(h + 1) * 128, :]
        w1 = np.asarray(inputs["W1"], f)[l]
        for k in range(2):
            blob[:, BF_OFF[("W1", l, k)]:BF_OFF[("W1", l, k)] + DFF] = \
                w1[k * 128:(k + 1) * 128, :]
        w2 = np.asarray(inputs["W2"], f)[l]
        for dk in range(8):
            blob[:, BF_OFF[("W2", l, dk)]:BF_OFF[("W2", l, dk)] + E] = \
                w2[dk * 128:(dk + 1) * 128, :]

    fblob_base = np.zeros((128, F_COLS), f)
    for l in range(LYR):
        for nm, w in (("bq", 2), ("bk", 2), ("b1", 8)):
            arr = np.asarray(inputs[nm], f)[l].reshape(w, 128).T
            fblob_base[:, F_OFF[(nm, l)]:F_OFF[(nm, l)] + w] = arr
    fblob_base[:, F_OFF["bemb_pp"]:F_OFF["bemb_pp"] + 2] = \
        np.asarray(inputs["b_emb"], f).reshape(2, 128).T
    fblob_base[:, F_OFF["Wp2"]:F_OFF["Wp2"] + 2] = \
        (np.asarray(inputs["lnf_g"], f)
         * np.asarray(inputs["W_proj"], f)[:, 0]).reshape(2, 128).T

    rows = np.zeros((1, R_COLS), f)
    for nm, src in (("ln1g", "ln1_g"), ("ln1b", "ln1_b"),
                    ("ln2g", "ln2_g"), ("ln2b", "ln2_b")):
        for l in range(LYR):
            rows[0, R_OFF[(nm, l)]:R_OFF[(nm, l)] + E] = \
                np.asarray(inputs[src], f)[l]
    rows[0, R_OFF["bemb_r"]:R_OFF["bemb_r"] + E] = \
        np.asarray(inputs["b_emb"], f)
    # final LN gain/bias folded into the projection column:
    # (z*g + b) @ Wp0 + bp0 = z @ (g*Wp0) + (b@Wp0 + bp0)
    wp0 = np.asarray(inputs["W_proj"], f)[:, 0]
    lnfg = np.asarray(inputs["lnf_g"], f)
    lnfb = np.asarray(inputs["lnf_b"], f)
    rows[0, R_OFF["bproj"]] = (np.asarray(inputs["b_proj"], f)[0]
                               + float(lnfb @ wp0))

    wrows = np.zeros((1, WR_COLS), bh)
    for nm, src in (("bv", "bv"), ("bo", "bo"), ("b2", "b2")):
        for l in range(LYR):
            wrows[0, WR_OFF[(nm, l)]:WR_OFF[(nm, l)] + E] = \
                np.asarray(inputs[src], f)[l]

    # 8 cores: 2 per batch.  Core 2b handles output positions [208, 256);
    # core 2b+1 gets inputs position-rolled by +ROLL so its local window
    # [208, 256) is global [160, 208).  Layer-1 is position-wise or
    # key-order-invariant, so the roll only permutes it.
    maps = []
    for b in range(B):
        dec0 = SCALE * np.exp(-td[b].reshape(NPOS, 8) / FACTOR)  # [q, chunk]
        x0 = x_enc[b, P0:P0 + NPOS, :]                           # [q, D]
        for roll in (0, ROLL):
            fblob = fblob_base.copy()
            dec = np.roll(dec0, roll, axis=0)
            for h in range(2):
                tile_ = np.repeat(dec[:, 4 * h:4 * h + 4].T, 32, axis=0)
                fblob[:, F_OFF[("D", h)]:F_OFF[("D", h)] + NPOS] = tile_
            xwa = np.zeros((D, 2 * NPOS), f)
            xwa[:, :NPOS] = np.roll(x0, roll, axis=0).T
            xwa[:, NPOS:] = np.asarray(inputs["W_emb"], f)
            maps.append({
                "blob": blob, "fblob": np.ascontiguousarray(fblob),
                "rows": rows, "wrows": wrows,
                "xw": np.ascontiguousarray(xwa),
            })
    return maps


def _run(in_maps, check_with_sim=False, check_with_hw=True, **kw):
    from concourse.bass_test_utils import run_kernel

    n = len(in_maps)
    out_like = {"out": np.zeros((OUTW, 1), np.float32)}
    res = run_kernel(
        lambda tc, outs, ins: chaos_kernel(tc, outs, ins),
        None,
        in_maps if n > 1 else in_maps[0],
        output_like=[out_like] * n if n > 1 else out_like,
        bass_type=tile.TileContext,
        num_cores=n,
        check_with_sim=check_with_sim,
        check_with_hw=check_with_hw,
        trace_sim=False,
        **kw,
    )
    return res


def kernel(**inputs):
    in_maps = _make_in_maps(inputs)
    res = _run(in_maps)
    # core 2b+1 produced global [160, 208), core 2b produced [208, 256)
    out = np.stack([
        np.concatenate([
            list(res.results[2 * b + 1].values())[0].reshape(OUTW),
            list(res.results[2 * b].values())[0].reshape(OUTW),
        ]) for b in range(B)])
    return out.astype(np.float32)
